# revision 4
# baseline (speedup 1.0000x reference)
"""Trainium2 Bass kernel for ContinuousTimeAttention.

B=2, S=2048, HID=1024, NH=16, HD=64. Sharded over 8 NeuronCores:
core c handles batch b=c//4 and the 4 heads h0=(c%4)*4 .. h0+4.

Per-core program (all matmul operands fp16, fp32 PSUM accumulation):
  - QT/KT = (x @ W.T + b).T in [feature, seq] layout (feature on partitions)
  - V in natural [seq, feature] layout, packed per head with a ones column
    (the ones column produces the softmax denominator for free in the AV
    matmul)
  - decay cache: exp(-r * |t_i - t_j|) over the causal lower triangle,
    computed once (r shared across heads in the common case) and stored fp16
  - scores computed transposed, S^T[j, i] = K Q^T, tile by tile; multiplied
    by (1/8 * decay) on DVE; exp on ACT (no max subtraction: |scores| <= ~8
    so exp is safe in fp32); causal diagonal blocks masked post-exp on GPSIMD
  - AV: out[i, 0:65] = sum_j P^T[j,i] * [V | 1], normalized by the ones
    column via per-partition reciprocal scaling
  - output projection: partial = attn_out @ Wo[:, cols].T via a DMA-transposed
    attn_out; partials summed on the host (tensor-parallel over heads).
"""

import math
from contextlib import ExitStack

import numpy as np

import concourse.bass as bass
import concourse.mybir as mybir
import concourse.tile as tile
from concourse.bass_utils import run_bass_kernel_spmd

F16 = mybir.dt.float16
F32 = mybir.dt.float32
ALU = mybir.AluOpType
ACTF = mybir.ActivationFunctionType

B, S, HID, NH, HD = 2, 2048, 1024, 16, 64
NHL = 4          # heads per core
DL = NHL * HD    # 256 local feature dim
P = 128
ST = S // P      # 16 seq tiles
N_CORES = 8
VW = HD + 1      # 65: V columns per head incl. ones column

# decay cache column offsets: cache for j-tile jt covers i in [jt*128, S)
DC_OFF = [0] * ST
for _jt in range(1, ST):
    DC_OFF[_jt] = DC_OFF[_jt - 1] + (S - (_jt - 1) * P)
DC_TOT = DC_OFF[-1] + (S - (ST - 1) * P)  # 17408


def _split_drain_waits(nc, max_waits=1):
    """walrus in this env rejects instructions with >1 sync wait; move extra
    waits onto single-wait NOPs immediately before, on the same engine."""
    for bb in nc.main_func.blocks:
        new_list = []
        for ins in bb.instructions:
            si = getattr(ins, "sync_info", None)
            waits = list(si.on_wait) if si and si.on_wait else []
            if len(waits) > max_waits:
                extra, keep = waits[:-max_waits], waits[-max_waits:]
                for w in extra:
                    nop = mybir.InstNoOp(
                        name=nc.get_next_instruction_name(), ins=[], outs=[]
                    )
                    nop.engine = ins.engine
                    nop.sync_info = mybir.SyncInfo(on_wait=[w], on_update=[])
                    new_list.append(nop)
                si.on_wait = keep
            new_list.append(ins)
        bb.instructions[:] = new_list


def _emit_body(nc, tc, ctx, io, shared_r):
    """Emit one full forward pass. io: dict of dram parameter handles."""
    sync = nc.sync

    const = ctx.enter_context(tc.tile_pool(name="const", bufs=1))

    trow_t = const.tile([P, S], F32)
    sync.dma_start(out=trow_t[:], in_=io["trow"][:])
    tcol_t = const.tile([P, ST], F32)
    sync.dma_start(out=tcol_t[:], in_=io["tcol"][:])
    negr_t = const.tile([P, NHL], F32)
    sync.dma_start(out=negr_t[:], in_=io["negr"][:])
    mask_t = const.tile([P, P], F16)
    sync.dma_start(out=mask_t[:], in_=io["trimask"][:])
    bq_t = const.tile([P, 2], F32)
    sync.dma_start(out=bq_t[:], in_=io["bq"][:])
    bk_t = const.tile([P, 2], F32)
    sync.dma_start(out=bk_t[:], in_=io["bk"][:])
    bvb_t = const.tile([P, DL], F32)
    sync.dma_start(out=bvb_t[:], in_=io["bvb"][:])
    woT_t = const.tile([P, 2, HID], F16)
    for ft in range(2):
        sync.dma_start(out=woT_t[:, ft, :], in_=io["woT"][ft])

    qt_t = const.tile([P, 2, S], F16)     # Q^T  [feature, seq]
    kt_t = const.tile([P, 2, S], F16)
    v4_t = const.tile([P, ST, NHL * VW], F16)  # V packed per head + ones col
    aout_t = const.tile([P, ST, DL], F16)      # attention out, [seq, feature]
    dcache = const.tile([P, DC_TOT], F16)      # decay (or |dt| if !shared_r)
    # mul / u staging buffer, fp32, one full causal row of a j-tile
    mulpool = ctx.enter_context(tc.tile_pool(name="mul", bufs=3))

    # ---- phase 1: decay cache build (DVE + ACT; overlaps QKV on PE) ----
    for jt in range(ST):
        w = S - jt * P
        ub = mulpool.tile([P, S], F32, tag="mulbuf")
        db = mulpool.tile([P, S], F32, tag="dbuf")
        for c0 in range(0, w, 1024):
            cw = min(1024, w - c0)
            # d = t_i - t_j ; u = max(-d, d) = |t_i - t_j|
            nc.vector.tensor_scalar(
                out=db[:, c0 : c0 + cw],
                in0=trow_t[:, jt * P + c0 : jt * P + c0 + cw],
                scalar1=tcol_t[:, jt : jt + 1],
                scalar2=None,
                op0=ALU.subtract,
            )
            nc.vector.scalar_tensor_tensor(
                out=ub[:, c0 : c0 + cw],
                in0=db[:, c0 : c0 + cw],
                scalar=-1.0,
                in1=db[:, c0 : c0 + cw],
                op0=ALU.mult,
                op1=ALU.max,
            )
        if shared_r:
            # dcache = exp(-r * |dt|)
            nc.scalar.activation(
                dcache[:, DC_OFF[jt] : DC_OFF[jt] + w],
                ub[:, 0:w],
                ACTF.Exp,
                scale=negr_t[:, 0:1],
            )
        else:
            # dcache = |dt| ; per-head exp happens in the attention loop
            nc.scalar.activation(
                dcache[:, DC_OFF[jt] : DC_OFF[jt] + w], ub[:, 0:w], ACTF.Copy
            )

    # ---- phase 2: QKV projections ----
    with tc.tile_pool(name="xw", bufs=1) as xw, tc.tile_pool(
        name="qkv_psum", bufs=4, space="PSUM"
    ) as qkv_psum:
        xT_t = xw.tile([P, 8, S], F16)
        for dt in range(8):
            sync.dma_start(out=xT_t[:, dt, :], in_=io["xT"][dt])
        wq_t = xw.tile([P, 8, DL], F16)
        wk_t = xw.tile([P, 8, DL], F16)
        wv_t = xw.tile([P, 8, DL], F16)
        for dt in range(8):
            sync.dma_start(out=wq_t[:, dt, :], in_=io["wqT"][dt])
            sync.dma_start(out=wk_t[:, dt, :], in_=io["wkT"][dt])
            sync.dma_start(out=wv_t[:, dt, :], in_=io["wvT"][dt])

        # Q^T, K^T: [m=128 x 2, seq] ; stationary = W^T d-tile, moving = x^T
        for dst_t, w_t, b_t in ((qt_t, wq_t, bq_t), (kt_t, wk_t, bk_t)):
            for mt in range(2):
                for sc in range(0, S, 512):
                    ps = qkv_psum.tile([P, 512], F32, tag="qkv")
                    for dt in range(8):
                        nc.tensor.matmul(
                            ps[:],
                            w_t[:, dt, mt * P : (mt + 1) * P],
                            xT_t[:, dt, sc : sc + 512],
                            start=(dt == 0),
                            stop=(dt == 7),
                        )
                    nc.scalar.activation(
                        dst_t[:, mt, sc : sc + 512],
                        ps[:],
                        ACTF.Identity,
                        bias=b_t[:, mt : mt + 1],
                    )

        # V natural [seq, feature]; ones cols preset by memset
        nc.vector.memset(v4_t[:], 1.0)
        for st in range(ST):
            ps = qkv_psum.tile([P, DL], F32, tag="qkv")
            for dt in range(8):
                nc.tensor.matmul(
                    ps[:],
                    xT_t[:, dt, st * P : (st + 1) * P],
                    wv_t[:, dt, :],
                    start=(dt == 0),
                    stop=(dt == 7),
                )
            # add bias and pack [4x64] into the [4x65] per-head layout
            v_dst = v4_t[:, st, :].rearrange("p (h c) -> p h c", h=NHL)[:, :, 0:HD]
            nc.vector.tensor_tensor(
                out=v_dst,
                in0=ps[:].rearrange("p (h c) -> p h c", h=NHL),
                in1=bvb_t[:].rearrange("p (h c) -> p h c", h=NHL),
                op=ALU.add,
            )

    # ---- phase 3: attention per head ----
    with tc.tile_pool(name="attn", bufs=2) as attn, tc.tile_pool(
        name="s_psum", bufs=2, space="PSUM"
    ) as s_psum, tc.tile_pool(
        name="av_psum", bufs=4, space="PSUM"
    ) as av_psum, tc.tile_pool(name="recip", bufs=8) as recip_pool:
        for h in range(NHL):
            mt, prow = h // 2, (h % 2) * 64
            p_tiles = []
            for jt in range(ST):
                w = S - jt * P
                i0a = jt * P  # absolute start query index
                pt = attn.tile([P, w], F16, tag=f"P{jt}")
                mb = mulpool.tile([P, S], F32, tag="mulbuf")
                if not shared_r:
                    dk = attn.tile([P, w], F16, tag=f"D{jt}")
                    nc.scalar.activation(
                        dk[:, 0:w],
                        dcache[:, DC_OFF[jt] : DC_OFF[jt] + w],
                        ACTF.Exp,
                        scale=negr_t[:, h : h + 1],
                    )
                    dsrc, doff = dk, 0
                else:
                    dsrc, doff = dcache, DC_OFF[jt]
                for c0 in range(0, w, 1024):
                    cw = min(1024, w - c0)
                    ps = s_psum.tile([P, 1024], F32, tag="s")
                    for sub in range(0, cw, 512):
                        sw = min(512, cw - sub)
                        nc.tensor.matmul(
                            ps[:, sub : sub + sw],
                            kt_t[prow : prow + 64, mt, jt * P : (jt + 1) * P],
                            qt_t[prow : prow + 64, mt, i0a + c0 + sub : i0a + c0 + sub + sw],
                            start=True,
                            stop=True,
                        )
                    # mulbuf = (scores * 1/8) * decay
                    nc.vector.scalar_tensor_tensor(
                        out=mb[:, c0 : c0 + cw],
                        in0=ps[:, 0:cw],
                        scalar=1.0 / math.sqrt(HD),
                        in1=dsrc[:, doff + c0 : doff + c0 + cw],
                        op0=ALU.mult,
                        op1=ALU.mult,
                    )
                nc.scalar.activation(pt[:, 0:w], mb[:, 0:w], ACTF.Exp)
                # causal mask on the diagonal block (post-exp, 0/1 mask)
                nc.gpsimd.tensor_tensor(
                    out=pt[:, 0:P], in0=pt[:, 0:P], in1=mask_t[:], op=ALU.mult
                )
                p_tiles.append(pt)

            for it in range(ST):
                pav = av_psum.tile([P, VW], F32, tag="av")
                for jt in range(it + 1):
                    nc.tensor.matmul(
                        pav[:],
                        p_tiles[jt][:, (it - jt) * P : (it - jt + 1) * P],
                        v4_t[:, jt, h * VW : (h + 1) * VW],
                        start=(jt == 0),
                        stop=(jt == it),
                    )
                rc = recip_pool.tile([P, 1], F32)
                nc.vector.reciprocal(rc[:], pav[:, HD : HD + 1])
                nc.scalar.activation(
                    aout_t[:, it, h * HD : (h + 1) * HD],
                    pav[:, 0:HD],
                    ACTF.Copy,
                    scale=rc[:],
                )

    # ---- phase 4: transpose attn_out, output projection ----
    with tc.tile_pool(name="proj", bufs=1) as proj, tc.tile_pool(
        name="o_psum", bufs=4, space="PSUM"
    ) as o_psum, tc.tile_pool(name="ostage", bufs=4) as ostage:
        aoutT_t = proj.tile([P, 2, S], F16)  # [feature, seq]
        for st in range(ST):
            for ft in range(2):
                sync.dma_start(
                    out=aoutT_t[:, ft, st * P : (st + 1) * P],
                    in_=aout_t[:, st, ft * P : (ft + 1) * P],
                    transpose=True,
                )
        for st in range(ST):
            for nt in range(2):
                po = o_psum.tile([P, 512], F32, tag="o")
                for ft in range(2):
                    nc.tensor.matmul(
                        po[:],
                        aoutT_t[:, ft, st * P : (st + 1) * P],
                        woT_t[:, ft, nt * 512 : (nt + 1) * 512],
                        start=(ft == 0),
                        stop=(ft == 1),
                    )
                ob = ostage.tile([P, 512], F32)
                if (st + nt) % 2 == 0:
                    nc.scalar.activation(ob[:], po[:], ACTF.Copy)
                else:
                    nc.vector.tensor_copy(ob[:], po[:])
                sync.dma_start(
                    out=io["out"][st * P : (st + 1) * P, nt * 512 : (nt + 1) * 512],
                    in_=ob[:],
                )


def build_program(shared_r=True, reps=1):
    nc = bass.Bass()
    io = {
        "xT": nc.declare_dram_parameter("xT", [8, P, S], F16, isOutput=False),
        "wqT": nc.declare_dram_parameter("wqT", [8, P, DL], F16, isOutput=False),
        "wkT": nc.declare_dram_parameter("wkT", [8, P, DL], F16, isOutput=False),
        "wvT": nc.declare_dram_parameter("wvT", [8, P, DL], F16, isOutput=False),
        "woT": nc.declare_dram_parameter("woT", [2, P, HID], F16, isOutput=False),
        "bq": nc.declare_dram_parameter("bq", [P, 2], F32, isOutput=False),
        "bk": nc.declare_dram_parameter("bk", [P, 2], F32, isOutput=False),
        "bvb": nc.declare_dram_parameter("bvb", [P, DL], F32, isOutput=False),
        "trow": nc.declare_dram_parameter("trow", [P, S], F32, isOutput=False),
        "tcol": nc.declare_dram_parameter("tcol", [P, ST], F32, isOutput=False),
        "negr": nc.declare_dram_parameter("negr", [P, NHL], F32, isOutput=False),
        "trimask": nc.declare_dram_parameter("trimask", [P, P], F16, isOutput=False),
        "out": nc.declare_dram_parameter("out", [S, HID], F32, isOutput=True),
    }
    with tile.TileContext(nc) as tc, ExitStack() as ctx:
        if reps == 1:
            _emit_body(nc, tc, ctx, io, shared_r)
        else:
            with tc.For_i(0, reps, 1):
                with ExitStack() as loop_ctx:
                    _emit_body(nc, tc, loop_ctx, io, shared_r)
    _split_drain_waits(nc)
    return nc


def shard_inputs(x, time_deltas, Wq, bq, Wk, bk, Wv, bv, Wo, bo, time_decay):
    """Build the 8 per-core input maps."""
    tri = (np.arange(P)[:, None] <= np.arange(P)[None, :]).astype(np.float16)
    in_maps = []
    for c in range(N_CORES):
        b, hb = c // 4, c % 4
        rows = slice(hb * NHL * HD, (hb + 1) * NHL * HD)
        xT = np.ascontiguousarray(x[b].T).astype(np.float16).reshape(8, P, S)
        m = {
            "xT": xT,
            "wqT": np.ascontiguousarray(Wq[rows].T).astype(np.float16).reshape(8, P, DL),
            "wkT": np.ascontiguousarray(Wk[rows].T).astype(np.float16).reshape(8, P, DL),
            "wvT": np.ascontiguousarray(Wv[rows].T).astype(np.float16).reshape(8, P, DL),
            "woT": np.ascontiguousarray(Wo[:, rows].T).astype(np.float16).reshape(2, P, HID),
            "bq": np.ascontiguousarray(bq[rows].reshape(2, P).T).astype(np.float32),
            "bk": np.ascontiguousarray(bk[rows].reshape(2, P).T).astype(np.float32),
            "bvb": np.ascontiguousarray(
                np.broadcast_to(bv[rows], (P, DL))
            ).astype(np.float32),
            "trow": np.ascontiguousarray(
                np.broadcast_to(time_deltas[b], (P, S))
            ).astype(np.float32),
            "tcol": np.ascontiguousarray(time_deltas[b].reshape(ST, P).T).astype(
                np.float32
            ),
            "negr": np.ascontiguousarray(
                np.broadcast_to(-time_decay[hb * NHL : (hb + 1) * NHL], (P, NHL))
            ).astype(np.float32),
            "trimask": tri,
        }
        in_maps.append(m)
    return in_maps


_PROGRAMS = {}


def _get_program(shared_r, reps=1):
    key = (shared_r, reps)
    if key not in _PROGRAMS:
        _PROGRAMS[key] = build_program(shared_r=shared_r, reps=reps)
    return _PROGRAMS[key]


def kernel(x, time_deltas, Wq, bq, Wk, bk, Wv, bv, Wo, bo, time_decay, _reps=1):
    x = np.asarray(x, dtype=np.float32)
    time_deltas = np.asarray(time_deltas, dtype=np.float32)
    Wq, bq = np.asarray(Wq, np.float32), np.asarray(bq, np.float32)
    Wk, bk = np.asarray(Wk, np.float32), np.asarray(bk, np.float32)
    Wv, bv = np.asarray(Wv, np.float32), np.asarray(bv, np.float32)
    Wo, bo = np.asarray(Wo, np.float32), np.asarray(bo, np.float32)
    time_decay = np.asarray(time_decay, np.float32)

    shared_r = bool(np.all(time_decay == time_decay[0]))
    nc = _get_program(shared_r, _reps)
    in_maps = shard_inputs(
        x, time_deltas, Wq, bq, Wk, bk, Wv, bv, Wo, bo, time_decay
    )
    res = run_bass_kernel_spmd(nc, in_maps, list(range(N_CORES)))
    out = np.zeros((B, S, HID), dtype=np.float32)
    for c in range(N_CORES):
        out[c // 4] += res.results[c]["out"]
    out += bo[None, None, :]
    return out
